# revision 3
# baseline (speedup 1.0000x reference)
"""GCN2 Trainium2 kernel: 3-layer GCN + FC head with BatchNorm, 8-core data-parallel.

Self-contained: hardcodes shapes from the problem spec.
  x [256, 128, 65] f32, adj_mat [256, 256] f32, W1 [63, 512], b1 [512],
  W2 [512, 512], b2 [512], W3 [512, 1024], b3 [1024], fcW1 [1024, 512],
  fcb1 [512], gamma [512], beta [512], fcW2 [512, 1], fcb2 [1] -> out [256, 1]

Sharding: batch 256 -> 32 samples per core on 8 cores; weights/adj replicated.
BatchNorm batch stats all-reduced across cores (one tiny [1,1024] AllReduce).

Key structural choices (v2, DMA-lean rewrite):
  - No indirect DMA, no DMA transposes, no DRAM scratch: the scatter
    X[sid[j]] = F[j] is expressed as one-hot matmuls built on-chip with
    iota + is_equal (ids are unique per sample, so sum-scatter == scatter).
  - Layer chain picked so every matmul output feeds the next stage's
    lhsT/rhs with zero transposes:
      Xs   (node-major)   = OH^T F           [per-sample scatter]
      Y1T  (feat-major)   = Xs^T An          (An symmetric)
      X1   (node-major)   = relu(Y1T^T W1p)  [b1 via ones row]
      ZT   (feat-major)   = X1^T An
      X2T  (feat-major)   = relu(W2^T ZT + b2)
      r    (feat-major)   = X2T . An[g,:]    [DVE weighted free-reduce]
  - Final layer needs only row g_sid of X3: batched head over 32 samples,
    G3 feature-major, H sample-major; fcb1 dropped (cancelled by BN).
  - BN stats via ones-vector matmuls -> [1,1024] f32 AllReduce (4KB).
  - All HBM loads are plain f32 HWDGE DMAs with contiguous runs; casts to
    bf16 happen on-chip (DVE/ACT are far from saturated).
"""
import os
import sys

if "/opt/trn_rl_repo" not in sys.path:
    sys.path.insert(0, "/opt/trn_rl_repo")

import numpy as np

import concourse.bass as bass
import concourse.mybir as mybir
import concourse.tile as tile
from concourse import bacc, bass_utils
from concourse.masks import make_identity

N_CORES = 8
BATCH, NODE, SEQ, FEAT = 256, 256, 128, 63   # FEAT = feature_num - 1
H1, H2, H3, FC = 512, 512, 1024, 512
BN_EPS = 1e-5
LEAKY = 0.01

F32 = mybir.dt.float32
BF16 = mybir.dt.bfloat16
I32 = mybir.dt.int32
AX = mybir.AxisListType
OP = mybir.AluOpType
ACTF = mybir.ActivationFunctionType


def build_nc(S: int):
    """Build the SPMD kernel for S samples per core."""
    nc = bacc.Bacc("TRN2", target_bir_lowering=False, debug=False,
                   num_devices=N_CORES)

    x_d = nc.dram_tensor("x", [S, SEQ, FEAT + 2], F32, kind="ExternalInput").ap()
    adj_d = nc.dram_tensor("adj_mat", [NODE, NODE], F32, kind="ExternalInput").ap()
    W1_d = nc.dram_tensor("W1", [FEAT, H1], F32, kind="ExternalInput").ap()
    b1_d = nc.dram_tensor("b1", [H1], F32, kind="ExternalInput").ap()
    W2_d = nc.dram_tensor("W2", [H1, H2], F32, kind="ExternalInput").ap()
    b2_d = nc.dram_tensor("b2", [H2], F32, kind="ExternalInput").ap()
    W3_d = nc.dram_tensor("W3", [H2, H3], F32, kind="ExternalInput").ap()
    b3_d = nc.dram_tensor("b3", [H3], F32, kind="ExternalInput").ap()
    fcW1_d = nc.dram_tensor("fcW1", [H3, FC], F32, kind="ExternalInput").ap()
    fcb1_d = nc.dram_tensor("fcb1", [FC], F32, kind="ExternalInput").ap()
    gamma_d = nc.dram_tensor("gamma", [FC], F32, kind="ExternalInput").ap()
    beta_d = nc.dram_tensor("beta", [FC], F32, kind="ExternalInput").ap()
    fcW2_d = nc.dram_tensor("fcW2", [FC, 1], F32, kind="ExternalInput").ap()
    fcb2_d = nc.dram_tensor("fcb2", [1], F32, kind="ExternalInput").ap()
    out_d = nc.dram_tensor("out", [S, 1], F32, kind="ExternalOutput").ap()

    with tile.TileContext(nc) as tc:
        _body(nc, tc, S, x_d, adj_d, W1_d, b1_d, W2_d, b2_d, W3_d, b3_d,
              fcW1_d, gamma_d, beta_d, fcW2_d, fcb2_d, out_d)
    nc.compile()
    return nc


def _body(nc, tc, S, x_d, adj_d, W1_d, b1_d, W2_d, b2_d, W3_d, b3_d,
          fcW1_d, gamma_d, beta_d, fcW2_d, fcb2_d, out_d):
    stage = int(os.environ.get("BISECT_STAGE", "0"))
    with tc.tile_pool(name="const", bufs=1) as cp, \
         tc.tile_pool(name="work", bufs=3) as wp, \
         tc.tile_pool(name="psA", bufs=2, space="PSUM") as psA, \
         tc.tile_pool(name="psB", bufs=3, space="PSUM") as psB, \
         tc.tile_pool(name="dram", bufs=1, space="DRAM") as dp:

        # ---------------- input DMAs (plain f32, contiguous runs) -----------
        # early: adjacency + x + W1/biases (needed at loop start) on sync ring
        A0 = cp.tile([128, 2, NODE], F32)        # chunk c = rows 128c..128c+127
        nc.sync.dma_start(A0[:], adj_d.rearrange("(c p) n -> p c n", p=128))
        Fall = cp.tile([128, S, FEAT + 2], F32)  # [seq, sample, feat]
        nc.sync.dma_start(Fall[:], x_d.rearrange("b j f -> j b f"))
        W1f = cp.tile([FEAT, H1], F32)
        nc.sync.dma_start(W1f[:], W1_d[:])
        b2raw = cp.tile([4, 128], F32)
        nc.sync.dma_start(b2raw[:], b2_d.rearrange("(c p) -> c p", p=128))
        b3raw = cp.tile([8, 128], F32)
        nc.sync.dma_start(b3raw[:], b3_d.rearrange("(c p) -> c p", p=128))
        gam_r = cp.tile([1, FC], F32)
        nc.sync.dma_start(gam_r[:], gamma_d[None, :])
        bet_r = cp.tile([1, FC], F32)
        nc.sync.dma_start(bet_r[:], beta_d[None, :])
        fcW2r = cp.tile([1, FC], F32)
        nc.sync.dma_start(fcW2r[:], fcW2_d.rearrange("h 1 -> 1 h"))
        fcb2r = cp.tile([1, 1], F32)
        nc.sync.dma_start(fcb2r[:], fcb2_d[None, :])
        # bulk weights on the scalar HWDGE ring (overlap with loop)
        W2f = cp.tile([128, 4, H2], F32)
        nc.scalar.dma_start(W2f[:], W2_d.rearrange("(c p) h -> p c h", p=128))
        W3f = cp.tile([128, 4, H3], F32)
        nc.scalar.dma_start(W3f[:], W3_d.rearrange("(c p) h -> p c h", p=128))
        fcW1f = cp.tile([128, 8, FC], F32)
        nc.scalar.dma_start(fcW1f[:], fcW1_d.rearrange("(c p) h -> p c h", p=128))

        # ---------------- on-chip weight casts f32 -> bf16 ------------------
        # W1p K=96 layout: rows 0-62 = W1, row 64 = b1 (ones-row trick), rest 0
        W1p = cp.tile([96, H1], BF16)
        nc.vector.memset(W1p[:], 0.0)
        nc.vector.tensor_copy(W1p[0:FEAT, :], W1f[:])
        nc.gpsimd.dma_start(W1p[64:65, :], b1_d[None, :])   # tiny cast-DMA
        W2sb = cp.tile([128, 4, H2], BF16)
        nc.vector.tensor_copy(W2sb[:], W2f[:])
        W3sb = cp.tile([128, 4, H3], BF16)
        nc.scalar.activation(W3sb[:], W3f[:], ACTF.Copy)
        fcW1sb = cp.tile([128, 8, FC], BF16)
        nc.scalar.activation(fcW1sb[:], fcW1f[:], ACTF.Copy)

        ident = cp.tile([128, 128], F32)
        make_identity(nc, ident[:])

        # biases that must live feature-on-partition: PE transpose
        pb2 = psB.tile([128, 4], F32, tag="med")
        nc.tensor.transpose(pb2[:], b2raw[:], ident[0:4, 0:4])
        b2c = cp.tile([128, 4], F32)
        nc.vector.tensor_copy(b2c[:], pb2[:])
        pb3 = psB.tile([128, 8], F32, tag="med")
        nc.tensor.transpose(pb3[:], b3raw[:], ident[0:8, 0:8])
        b3c = cp.tile([128, 8], F32)
        nc.vector.tensor_copy(b3c[:], pb3[:])

        # ---------------- iotas / one-hot helpers ---------------------------
        iotaRowI = cp.tile([128, NODE], I32)
        nc.gpsimd.iota(iotaRowI[:], pattern=[[1, NODE]], base=0,
                       channel_multiplier=0)
        iotaRowF = cp.tile([128, NODE], F32)
        nc.vector.tensor_copy(iotaRowF[:], iotaRowI[:])
        iotaCNI = cp.tile([128, 2], I32)
        nc.gpsimd.iota(iotaCNI[:], pattern=[[128, 2]], base=0,
                       channel_multiplier=1)
        iotaCNF = cp.tile([128, 2], F32)
        nc.vector.tensor_copy(iotaCNF[:], iotaCNI[:])
        e127 = cp.tile([128, 1], F32)   # one-hot of partition 127
        nc.vector.tensor_scalar(out=e127[:], in0=iotaCNF[:, 0:1],
                                scalar1=float(SEQ - 1), scalar2=None,
                                op0=OP.is_equal)
        ones1 = cp.tile([1, 128], F32)
        nc.vector.memset(ones1[:], 1.0)
        onesS = cp.tile([S, 1], F32)
        nc.vector.memset(onesS[:], 1.0)
        Id32b = cp.tile([S, S], BF16)
        make_identity(nc, Id32b[:])

        # ---------------- adjacency normalization ---------------------------
        # An = diag(dis) (A + I) diag(dis),  dis = 1/sqrt(rowsum(A) + 1)
        dis = cp.tile([128, 2], F32)
        rs = cp.tile([128, 2], F32)
        for c in range(2):
            nc.vector.tensor_reduce(rs[:, c:c + 1], A0[:, c, :], axis=AX.X, op=OP.add)
        sq = cp.tile([128, 2], F32)
        nc.scalar.activation(sq[:], rs[:], ACTF.Sqrt, bias=1.0)
        nc.vector.reciprocal(dis[:], sq[:])
        dis2 = cp.tile([128, 2], F32)
        nc.vector.tensor_tensor(dis2[:], dis[:], dis[:], op=OP.mult)
        # C = diag(dis) A  (row scaling)
        Csc = cp.tile([128, 2, NODE], F32)
        for c in range(2):
            nc.vector.tensor_scalar_mul(Csc[:, c, :], A0[:, c, :],
                                        dis[:, c:c + 1])
        # An = diag(dis) C^T  (+ diag(dis^2))
        Anb = cp.tile([128, 2, NODE], BF16)      # normalized adjacency, bf16
        for cd in range(2):      # destination row chunk
            for cs in range(2):  # source row chunk
                pT = psB.tile([128, 128], F32, tag="med")
                nc.tensor.transpose(pT[:], Csc[:, cs, 128 * cd:128 * (cd + 1)],
                                    ident[:])
                nc.scalar.activation(Anb[:, cd, 128 * cs:128 * (cs + 1)], pT[:],
                                     ACTF.Copy, scale=dis[:, cd:cd + 1])
        diagb = cp.tile([128, 2, NODE], BF16)
        for c in range(2):
            nc.gpsimd.affine_select(
                out=diagb[:, c, :], in_=dis2[:, c:c + 1].to_broadcast([128, NODE]),
                pattern=[[-1, NODE]], compare_op=OP.is_equal, fill=0.0,
                base=128 * c, channel_multiplier=1)
            nc.vector.tensor_tensor(Anb[:, c, :], Anb[:, c, :], diagb[:, c, :],
                                    op=OP.add)

        if stage == 1:
            nc.sync.dma_start(out_d.rearrange("b 1 -> 1 b"), Anb[0:1, 0, 0:S])
            return

        # ---------------- station ids, g-row one-hots ------------------------
        sidF = cp.tile([128, S], F32)
        nc.vector.tensor_copy(sidF[:], Fall[:, :, FEAT:FEAT + 1].rearrange("p b 1 -> p b"))
        # gsidB[p, b] = sid[b, SEQ-1]  (broadcast row 127 to all partitions)
        pgs = psB.tile([128, S], F32, tag="med")
        nc.tensor.matmul(pgs[:], lhsT=e127[:, 0:1].to_broadcast([128, 128]),
                         rhs=sidF[:], start=True, stop=True)
        gsidB = cp.tile([128, S], F32)
        nc.vector.tensor_copy(gsidB[:], pgs[:])
        gOH = cp.tile([128, 2, S], BF16)         # node-major one-hot of g_sid
        for c in range(2):
            nc.vector.tensor_scalar(out=gOH[:, c, :], in0=gsidB[:],
                                    scalar1=iotaCNF[:, c:c + 1], scalar2=None,
                                    op0=OP.is_equal)
        # AnRows[b, :] = An[g_b, :]
        pAR = psB.tile([S, NODE], F32, tag="med")
        for c in range(2):
            nc.tensor.matmul(pAR[:], lhsT=gOH[:, c, :], rhs=Anb[:, c, :],
                             start=(c == 0), stop=(c == 1))
        AnRowsB = cp.tile([S, NODE], BF16)
        nc.vector.tensor_copy(AnRowsB[:], pAR[:])

        if stage == 2:
            nc.sync.dma_start(out_d.rearrange("b 1 -> 1 b"), AnRowsB[0:1, 0:S])
            return

        R = cp.tile([128, 4, S], F32)    # r vectors, feature-major
        if stage:
            nc.vector.memset(R[:], 0.0)

        # ---------------- per-sample pipeline -------------------------------
        for b in range(S):
            # one-hot OH[j, m] = (sid[b, j] == m), j on partitions
            OH = wp.tile([128, NODE], BF16, tag="OH")
            nc.vector.tensor_scalar(out=OH[:], in0=iotaRowF[:],
                                    scalar1=sidF[:, b:b + 1], scalar2=None,
                                    op0=OP.is_equal)
            Fb = wp.tile([128, FEAT], BF16, tag="Fb")
            nc.scalar.activation(Fb[:], Fall[:, b, 0:FEAT], ACTF.Copy)

            # scatter: Xs[m, f] = sum_j OH[j, m] F[j, f]  (node-major)
            pXs = psB.tile([128, 2, FEAT], F32, tag="med")
            for c in range(2):
                nc.tensor.matmul(pXs[:, c, :], lhsT=OH[:, 128 * c:128 * (c + 1)],
                                 rhs=Fb[:], start=True, stop=True)
            Xsb = wp.tile([128, 2, FEAT], BF16, tag="Xsb")
            nc.vector.tensor_copy(Xsb[:], pXs[:])

            # L1 graph-mult: Y1T[f, n] = sum_m Xs[m, f] An[m, n] (feat-major)
            pY1 = psB.tile([96, NODE], F32, tag="med")
            for c in range(2):
                nc.tensor.matmul(pY1[0:FEAT, :], lhsT=Xsb[:, c, :],
                                 rhs=Anb[:, c, :], start=(c == 0), stop=(c == 1))
            Y1Tb = wp.tile([96, NODE], BF16, tag="Y1T")
            nc.vector.memset(Y1Tb[32:64, :], 0.0)
            nc.vector.memset(Y1Tb[64:96, :], 0.0)
            nc.scalar.activation(Y1Tb[0:FEAT, :], pY1[0:FEAT, :], ACTF.Copy)
            nc.gpsimd.memset(Y1Tb[64:65, :], 1.0)

            if stage == 31:
                if b == S - 1:
                    nc.sync.dma_start(out_d.rearrange("b 1 -> 1 b"), Y1Tb[0:1, 0:S])
                continue

            # W1 (+b1 via ones row), relu -> X1 node-major [128, 2, 512]
            pX1 = psA.tile([128, 2, H1], F32, tag="big")
            for t in range(2):
                nc.tensor.matmul(pX1[:, t, :], lhsT=Y1Tb[:, 128 * t:128 * (t + 1)],
                                 rhs=W1p[:], start=True, stop=True)
            X1b = wp.tile([128, 2, H1], BF16, tag="X1")
            nc.scalar.activation(X1b[:, 0, :], pX1[:, 0, :], ACTF.Relu)
            nc.vector.tensor_scalar_max(X1b[:, 1, :], pX1[:, 1, :], 0.0)

            if stage == 32:
                if b == S - 1:
                    nc.sync.dma_start(out_d.rearrange("b 1 -> 1 b"), X1b[0:1, 0, 0:S])
                continue

            # L2 graph-mult, feature-major out: ZT[h, n] = sum_m X1[m, h] An[m, n]
            pZT = psA.tile([128, 4, NODE], F32, tag="big")
            for mb in range(4):
                for k in range(2):
                    nc.tensor.matmul(pZT[:, mb, :],
                                     lhsT=X1b[:, k, 128 * mb:128 * (mb + 1)],
                                     rhs=Anb[:, k, :],
                                     start=(k == 0), stop=(k == 1))
            ZTb = wp.tile([128, 4, NODE], BF16, tag="ZT")
            nc.scalar.activation(ZTb[:, 0:2, :], pZT[:, 0:2, :], ACTF.Copy)
            nc.vector.tensor_copy(ZTb[:, 2:4, :], pZT[:, 2:4, :])

            if stage == 33:
                if b == S - 1:
                    nc.sync.dma_start(out_d.rearrange("b 1 -> 1 b"), ZTb[0:1, 0, 0:S])
                continue

            # W2 + b2, relu -> X2T feature-major [128, 4, 256]
            pX2 = psA.tile([128, 4, NODE], F32, tag="big")
            for mb in range(4):
                for c in range(4):
                    nc.tensor.matmul(pX2[:, mb, :],
                                     lhsT=W2sb[:, c, 128 * mb:128 * (mb + 1)],
                                     rhs=ZTb[:, c, :],
                                     start=(c == 0), stop=(c == 3))
            X2b = wp.tile([128, 4, NODE], BF16, tag="X2")
            for mb in range(4):
                if mb < 2:
                    nc.scalar.activation(X2b[:, mb, :], pX2[:, mb, :], ACTF.Relu,
                                         bias=b2c[:, mb:mb + 1])
                else:
                    nc.vector.tensor_scalar(
                        out=X2b[:, mb, :], in0=pX2[:, mb, :],
                        scalar1=b2c[:, mb:mb + 1], scalar2=0.0,
                        op0=OP.add, op1=OP.max)

            if stage == 34:
                if b == S - 1:
                    nc.sync.dma_start(out_d.rearrange("b 1 -> 1 b"), X2b[0:1, 0, 0:S])
                continue

            # broadcast An[g_b, :] to all partitions
            pB = psB.tile([128, NODE], F32, tag="med")
            nc.tensor.matmul(pB[:], lhsT=Id32b[:, b:b + 1].to_broadcast([S, 128]),
                             rhs=AnRowsB[:], start=True, stop=True)
            ARb = wp.tile([128, NODE], BF16, tag="ARb")
            nc.vector.tensor_copy(ARb[:], pB[:])

            # r[h] = sum_n X2T[h, n] * An[g_b, n]
            junk = wp.tile([128, NODE], BF16, tag="junk")
            for m in range(4):
                nc.vector.scalar_tensor_tensor(
                    out=junk[:], in0=X2b[:, m, :], scalar=1.0, in1=ARb[:],
                    op0=OP.mult, op1=OP.mult, accum_out=R[:, m, b:b + 1])

        if stage == 3 or (30 < stage < 40):
            nc.sync.dma_start(out_d.rearrange("b 1 -> 1 b"), R[0:1, 0, 0:S])
            return

        # ---------------- batched head --------------------------------------
        Rbb = cp.tile([128, 4, S], BF16)
        nc.vector.tensor_copy(Rbb[:], R[:])
        # G3 = relu(W3^T r + b3), feature-major [128, 8, S]
        G3 = cp.tile([128, 8, S], BF16)
        for mb in range(8):
            pG = psB.tile([128, S], F32, tag="med")
            for c in range(4):
                nc.tensor.matmul(pG[:], lhsT=W3sb[:, c, 128 * mb:128 * (mb + 1)],
                                 rhs=Rbb[:, c, :], start=(c == 0), stop=(c == 3))
            nc.scalar.activation(G3[:, mb, :], pG[:], ACTF.Relu,
                                 bias=b3c[:, mb:mb + 1])

        # H = G3^T fcW1, sample-major [S, 512] (fcb1 cancelled by BN)
        pH = psB.tile([S, FC], F32, tag="med")
        for c in range(8):
            nc.tensor.matmul(pH[:], lhsT=G3[:, c, :], rhs=fcW1sb[:, c, :],
                             start=(c == 0), stop=(c == 7))
        H = cp.tile([S, FC], F32)
        nc.scalar.activation(H[:], pH[:], ACTF.Identity)
        Hsq = cp.tile([S, FC], F32)
        nc.scalar.activation(Hsq[:], H[:], ACTF.Square)

        # local BN stats: [1, 0:512] = sum_b H, [1, 512:1024] = sum_b H^2
        pS1 = psB.tile([1, FC], F32, tag="med")
        nc.tensor.matmul(pS1[:], lhsT=onesS[:], rhs=H[:], start=True, stop=True)
        pS2 = psB.tile([1, FC], F32, tag="med")
        nc.tensor.matmul(pS2[:], lhsT=onesS[:], rhs=Hsq[:], start=True, stop=True)
        stats = cp.tile([1, 2 * FC], F32)
        nc.vector.tensor_copy(stats[:, 0:FC], pS1[:])
        nc.vector.tensor_copy(stats[:, FC:2 * FC], pS2[:])

        if stage == 4:
            nc.sync.dma_start(out_d.rearrange("b 1 -> 1 b"), stats[0:1, 0:S])
            return

        cc_in = dp.tile([1, 2 * FC], F32)
        cc_out = dp.tile([1, 2 * FC], F32)
        nc.sync.dma_start(cc_in[:], stats[:])
        nc.gpsimd.collective_compute(
            "AllReduce", OP.add, replica_groups=[list(range(N_CORES))],
            ins=[cc_in.opt()], outs=[cc_out.opt()])
        statsG = cp.tile([1, 2 * FC], F32)
        nc.sync.dma_start(statsG[:], cc_out[:])

        if stage == 5:
            nc.sync.dma_start(out_d.rearrange("b 1 -> 1 b"), statsG[0:1, 0:S])
            return

        # BN math on [1, 512] rows
        inv_n = 1.0 / (S * N_CORES)
        mean = cp.tile([1, FC], F32)
        nc.vector.tensor_scalar_mul(mean[:], statsG[:, 0:FC], inv_n)
        ex2 = cp.tile([1, FC], F32)
        nc.vector.tensor_scalar_mul(ex2[:], statsG[:, FC:2 * FC], inv_n)
        var = cp.tile([1, FC], F32)
        nc.vector.tensor_tensor(var[:], mean[:], mean[:], op=OP.mult)
        nc.vector.tensor_tensor(var[:], ex2[:], var[:], op=OP.subtract)
        epsc = cp.tile([1, 1], F32)
        nc.gpsimd.memset(epsc[:], BN_EPS)
        sd = cp.tile([1, FC], F32)
        nc.scalar.activation(sd[:], var[:], ACTF.Sqrt, bias=epsc[:, 0:1])
        rstd = cp.tile([1, FC], F32)
        nc.vector.reciprocal(rstd[:], sd[:])
        scl = cp.tile([1, FC], F32)
        nc.vector.tensor_tensor(scl[:], gam_r[:], rstd[:], op=OP.mult)
        sft = cp.tile([1, FC], F32)
        nc.vector.tensor_tensor(sft[:], mean[:], scl[:], op=OP.mult)
        nc.vector.tensor_tensor(sft[:], bet_r[:], sft[:], op=OP.subtract)

        # broadcast scl/sft/fcW2/fcb2 rows to S partitions via ones-matmul
        def _brc(row, n):
            p = psB.tile([S, n], F32, tag="med")
            nc.tensor.matmul(p[:], lhsT=ones1[:, 0:S], rhs=row[:],
                             start=True, stop=True)
            t = cp.tile([S, n], F32)
            nc.vector.tensor_copy(t[:], p[:])
            return t

        sclB = _brc(scl, FC)
        sftB = _brc(sft, FC)
        fcW2B = _brc(fcW2r, FC)
        fcb2B = _brc(fcb2r, 1)

        # normalize + leaky relu + final dot + sigmoid
        Hn = cp.tile([S, FC], F32)
        nc.vector.tensor_tensor(Hn[:], H[:], sclB[:], op=OP.mult)
        nc.vector.tensor_tensor(Hn[:], Hn[:], sftB[:], op=OP.add)
        Hl = cp.tile([S, FC], F32)
        nc.vector.scalar_tensor_tensor(
            out=Hl[:], in0=Hn[:], scalar=LEAKY, in1=Hn[:],
            op0=OP.mult, op1=OP.max)
        junk2 = cp.tile([S, FC], F32)
        ocol = cp.tile([S, 1], F32)
        nc.vector.scalar_tensor_tensor(
            out=junk2[:], in0=Hl[:], scalar=1.0, in1=fcW2B[:],
            op0=OP.mult, op1=OP.mult, accum_out=ocol[:])
        osig = cp.tile([S, 1], F32)
        nc.scalar.activation(osig[:], ocol[:], ACTF.Sigmoid,
                             bias=fcb2B[:, 0:1])
        nc.sync.dma_start(out_d[:], osig[:])


_NC_CACHE = {}
_LAST_RESULT = None


def _get_nc(S: int):
    if S not in _NC_CACHE:
        _NC_CACHE[S] = build_nc(S)
    return _NC_CACHE[S]


def kernel(**inputs) -> np.ndarray:
    S = BATCH // N_CORES
    nc = _get_nc(S)
    full_x = np.ascontiguousarray(inputs["x"], dtype=np.float32)
    shared = {}
    for k in ("adj_mat", "W1", "b1", "W2", "b2", "W3", "b3", "fcW1", "fcb1",
              "gamma", "beta", "fcW2", "fcb2"):
        shared[k] = np.ascontiguousarray(inputs[k], dtype=np.float32)
    in_maps = []
    for c in range(N_CORES):
        m = dict(shared)
        m["x"] = np.ascontiguousarray(full_x[c * S:(c + 1) * S])
        in_maps.append(m)
    res = bass_utils.run_bass_kernel_spmd(
        nc, in_maps, core_ids=list(range(N_CORES)))
    global _LAST_RESULT
    _LAST_RESULT = res
    out = np.concatenate([res.results[c]["out"] for c in range(N_CORES)], axis=0)
    return out.astype(np.float32)


if __name__ == "__main__":
    print("building...")
    nc = _get_nc(BATCH // N_CORES)
    print("built ok")


# revision 9
# speedup vs baseline: 1.0600x; 1.0600x over previous
"""GCN2 Trainium2 kernel: 3-layer GCN + FC head with BatchNorm, 8-core data-parallel.

Self-contained: hardcodes shapes from the problem spec.
  x [256, 128, 65] f32, adj_mat [256, 256] f32, W1 [63, 512], b1 [512],
  W2 [512, 512], b2 [512], W3 [512, 1024], b3 [1024], fcW1 [1024, 512],
  fcb1 [512], gamma [512], beta [512], fcW2 [512, 1], fcb2 [1] -> out [256, 1]

Sharding: batch 256 -> 32 samples per core on 8 cores; weights/adj replicated.
BatchNorm batch stats all-reduced across cores (one tiny [128,8] AllReduce).

v3 structure (DMA-lean + PE-dense):
  - No indirect DMA / DMA transposes / DRAM scratch: scatter via on-chip
    one-hot matmuls (iota + is_equal; ids unique per sample).
  - Zero-transpose layer chain:
      Xs (node-major) = OH^T F ; Y1T (feat-major) = Xs^T An ;
      X1 (node-major) = relu(Y1T^T W1p) ; ZT (feat-major) = X1^T An ;
      X2T (feat-major) = relu(W2^T ZT + b2) ; r = X2T . An[g,:] (DVE).
  - Samples processed in PAIRS so W2/ZT-consuming matmuls stream N=512
    (weight loads fully hidden; keeps the PE HAM-warm).
  - All PSUM tiles <= 1 bank, single deep pool for lookahead.
  - Evictions spread over Scalar/Vector/GpSimd; fcb1 dropped (BN cancels).
  - Feature-major head + per-partition BN math (no broadcasts, fast
    reciprocal on [128,4]); Lrelu fused via activation alpha.
"""
import os
import sys

if "/opt/trn_rl_repo" not in sys.path:
    sys.path.insert(0, "/opt/trn_rl_repo")

import numpy as np

import concourse.bass as bass
import concourse.mybir as mybir
import concourse.tile as tile
from concourse import bacc, bass_utils
from concourse.masks import make_identity

N_CORES = 8
BATCH, NODE, SEQ, FEAT = 256, 256, 128, 63   # FEAT = feature_num - 1
H1, H2, H3, FC = 512, 512, 1024, 512
BN_EPS = 1e-5
LEAKY = 0.01

F32 = mybir.dt.float32
BF16 = mybir.dt.bfloat16
I32 = mybir.dt.int32
AX = mybir.AxisListType
OP = mybir.AluOpType
ACTF = mybir.ActivationFunctionType


def build_nc(S: int):
    """Build the SPMD kernel for S samples per core."""
    nc = bacc.Bacc("TRN2", target_bir_lowering=False, debug=False,
                   num_devices=N_CORES)

    x_d = nc.dram_tensor("x", [S, SEQ, FEAT + 2], F32, kind="ExternalInput").ap()
    adj_d = nc.dram_tensor("adj_mat", [NODE, NODE], F32, kind="ExternalInput").ap()
    W1_d = nc.dram_tensor("W1", [FEAT, H1], F32, kind="ExternalInput").ap()
    b1_d = nc.dram_tensor("b1", [H1], F32, kind="ExternalInput").ap()
    W2_d = nc.dram_tensor("W2", [H1, H2], F32, kind="ExternalInput").ap()
    b2_d = nc.dram_tensor("b2", [H2], F32, kind="ExternalInput").ap()
    W3_d = nc.dram_tensor("W3", [H2, H3], F32, kind="ExternalInput").ap()
    b3_d = nc.dram_tensor("b3", [H3], F32, kind="ExternalInput").ap()
    fcW1_d = nc.dram_tensor("fcW1", [H3, FC], F32, kind="ExternalInput").ap()
    fcb1_d = nc.dram_tensor("fcb1", [FC], F32, kind="ExternalInput").ap()
    gamma_d = nc.dram_tensor("gamma", [FC], F32, kind="ExternalInput").ap()
    beta_d = nc.dram_tensor("beta", [FC], F32, kind="ExternalInput").ap()
    fcW2_d = nc.dram_tensor("fcW2", [FC, 1], F32, kind="ExternalInput").ap()
    fcb2_d = nc.dram_tensor("fcb2", [1], F32, kind="ExternalInput").ap()
    out_d = nc.dram_tensor("out", [S, 1], F32, kind="ExternalOutput").ap()

    with tile.TileContext(nc) as tc:
        _body(nc, tc, S, x_d, adj_d, W1_d, b1_d, W2_d, b2_d, W3_d, b3_d,
              fcW1_d, gamma_d, beta_d, fcW2_d, fcb2_d, out_d)
    nc.compile()
    return nc


def _body(nc, tc, S, x_d, adj_d, W1_d, b1_d, W2_d, b2_d, W3_d, b3_d,
          fcW1_d, gamma_d, beta_d, fcW2_d, fcb2_d, out_d):
    stage = int(os.environ.get("BISECT_STAGE", "0"))
    with tc.tile_pool(name="const", bufs=1) as cp, \
         tc.tile_pool(name="work", bufs=3) as wp, \
         tc.tile_pool(name="ps", bufs=7, space="PSUM") as ps, \
         tc.tile_pool(name="dram", bufs=1, space="DRAM") as dp:

        # ---------------- input DMAs (plain f32, contiguous runs) -----------
        A0 = cp.tile([128, 2, NODE], F32)        # chunk c = rows 128c..128c+127
        nc.sync.dma_start(A0[:], adj_d.rearrange("(c p) n -> p c n", p=128))
        Fall = cp.tile([128, S, FEAT + 2], F32)  # [seq, sample, feat]
        nc.sync.dma_start(Fall[:], x_d.rearrange("b j f -> j b f"))
        W1f = cp.tile([FEAT, H1], F32)
        nc.sync.dma_start(W1f[:], W1_d[:])
        b2raw = cp.tile([4, 128], F32)
        nc.sync.dma_start(b2raw[:], b2_d.rearrange("(c p) -> c p", p=128))
        b3raw = cp.tile([8, 128], F32)
        nc.sync.dma_start(b3raw[:], b3_d.rearrange("(c p) -> c p", p=128))
        gbraw = cp.tile([4, 3, 128], F32)        # gamma | beta | fcW2
        nc.sync.dma_start(gbraw[:, 0, :], gamma_d.rearrange("(c p) -> c p", p=128))
        nc.sync.dma_start(gbraw[:, 1, :], beta_d.rearrange("(c p) -> c p", p=128))
        nc.sync.dma_start(gbraw[:, 2, :], fcW2_d.rearrange("(c p) 1 -> c p", p=128))
        fcb2r = cp.tile([1, 1], F32)
        nc.sync.dma_start(fcb2r[:], fcb2_d[None, :])
        # bulk weights on the scalar HWDGE ring (overlap with loop)
        W2f = cp.tile([128, 4, H2], F32)
        nc.scalar.dma_start(W2f[:], W2_d.rearrange("(c p) h -> p c h", p=128))
        W3f = cp.tile([128, 4, H3], F32)
        nc.scalar.dma_start(W3f[:], W3_d.rearrange("(c p) h -> p c h", p=128))
        fcW1f = cp.tile([128, 8, FC], F32)
        nc.scalar.dma_start(fcW1f[:], fcW1_d.rearrange("(c p) h -> p c h", p=128))

        # ---------------- on-chip weight casts f32 -> bf16 ------------------
        W1p = cp.tile([96, H1], BF16)
        nc.vector.memset(W1p[:], 0.0)
        nc.vector.tensor_copy(W1p[0:FEAT, :], W1f[:])
        nc.gpsimd.dma_start(W1p[64:65, :], b1_d[None, :])   # tiny cast-DMA
        W2sb = cp.tile([128, 4, H2], BF16)
        nc.vector.tensor_copy(W2sb[:], W2f[:])
        W3sb = cp.tile([128, 4, H3], BF16)
        nc.scalar.activation(W3sb[:], W3f[:], ACTF.Copy)
        fcW1sb = cp.tile([128, 8, FC], BF16)
        nc.scalar.activation(fcW1sb[:], fcW1f[:], ACTF.Copy)

        ident = cp.tile([128, 128], F32)
        make_identity(nc, ident[:])

        # feature-on-partition constants via PE transpose
        def _tr(raw, n, dtype):
            p = ps.tile([128, n], F32, tag="w")
            nc.tensor.transpose(p[:], raw, ident[0:n, 0:n])
            t = cp.tile([128, n], dtype)
            nc.vector.tensor_copy(t[:], p[:])
            return t

        b2c = _tr(b2raw[:], 4, F32)
        b3c = _tr(b3raw[:], 8, F32)
        gamc = _tr(gbraw[:, 0, :], 4, F32)
        betc = _tr(gbraw[:, 1, :], 4, F32)
        fcW2c = _tr(gbraw[:, 2, :], 4, BF16)

        # ---------------- iotas / one-hot helpers ---------------------------
        iotaRowI = cp.tile([128, NODE], I32)
        nc.gpsimd.iota(iotaRowI[:], pattern=[[1, NODE]], base=0,
                       channel_multiplier=0)
        iotaRowF = cp.tile([128, NODE], F32)
        nc.vector.tensor_copy(iotaRowF[:], iotaRowI[:])
        iotaCNI = cp.tile([128, 2], I32)
        nc.gpsimd.iota(iotaCNI[:], pattern=[[128, 2]], base=0,
                       channel_multiplier=1)
        iotaCNF = cp.tile([128, 2], F32)
        nc.vector.tensor_copy(iotaCNF[:], iotaCNI[:])
        e127 = cp.tile([128, 1], F32)   # one-hot of partition 127
        nc.vector.tensor_scalar(out=e127[:], in0=iotaCNF[:, 0:1],
                                scalar1=float(SEQ - 1), scalar2=None,
                                op0=OP.is_equal)
        Id32b = cp.tile([S, S], BF16)
        make_identity(nc, Id32b[:])
        epsc = cp.tile([128, 1], F32)
        nc.gpsimd.memset(epsc[:], BN_EPS)

        # ---------------- adjacency normalization ---------------------------
        # An = diag(dis) (A + I) diag(dis),  dis = 1/sqrt(rowsum(A) + 1)
        dis = cp.tile([128, 2], F32)
        rs = cp.tile([128, 2], F32)
        for c in range(2):
            nc.vector.tensor_reduce(rs[:, c:c + 1], A0[:, c, :], axis=AX.X, op=OP.add)
        sq = cp.tile([128, 2], F32)
        nc.scalar.activation(sq[:], rs[:], ACTF.Sqrt, bias=1.0)
        nc.vector.reciprocal(dis[:], sq[:])
        dis2 = cp.tile([128, 2], F32)
        nc.vector.tensor_tensor(dis2[:], dis[:], dis[:], op=OP.mult)
        Csc = cp.tile([128, 2, NODE], F32)
        for c in range(2):
            nc.vector.tensor_scalar_mul(Csc[:, c, :], A0[:, c, :],
                                        dis[:, c:c + 1])
        Anb = cp.tile([128, 2, NODE], BF16)      # normalized adjacency, bf16
        for cd in range(2):      # destination row chunk
            for cs in range(2):  # source row chunk
                pT = ps.tile([128, 128], F32, tag="w")
                nc.tensor.transpose(pT[:], Csc[:, cs, 128 * cd:128 * (cd + 1)],
                                    ident[:])
                nc.scalar.activation(Anb[:, cd, 128 * cs:128 * (cs + 1)], pT[:],
                                     ACTF.Copy, scale=dis[:, cd:cd + 1])
        diagb = cp.tile([128, 2, NODE], BF16)
        for c in range(2):
            nc.gpsimd.affine_select(
                out=diagb[:, c, :], in_=dis2[:, c:c + 1].to_broadcast([128, NODE]),
                pattern=[[-1, NODE]], compare_op=OP.is_equal, fill=0.0,
                base=128 * c, channel_multiplier=1)
            nc.vector.tensor_tensor(Anb[:, c, :], Anb[:, c, :], diagb[:, c, :],
                                    op=OP.add)

        if stage == 1:
            nc.sync.dma_start(out_d.rearrange("b 1 -> 1 b"), Anb[0:1, 0, 0:S])
            return

        # ---------------- station ids, g-row one-hots ------------------------
        sidF = cp.tile([128, S], F32)
        nc.vector.tensor_copy(sidF[:], Fall[:, :, FEAT:FEAT + 1].rearrange("p b 1 -> p b"))
        pgs = ps.tile([128, S], F32, tag="w")
        nc.tensor.matmul(pgs[:], lhsT=e127[:, 0:1].to_broadcast([128, 128]),
                         rhs=sidF[:], start=True, stop=True)
        gsidB = cp.tile([128, S], F32)
        nc.vector.tensor_copy(gsidB[:], pgs[:])
        gOH = cp.tile([128, 2, S], BF16)         # node-major one-hot of g_sid
        for c in range(2):
            nc.vector.tensor_scalar(out=gOH[:, c, :], in0=gsidB[:],
                                    scalar1=iotaCNF[:, c:c + 1], scalar2=None,
                                    op0=OP.is_equal)
        pAR = ps.tile([S, NODE], F32, tag="w")
        for c in range(2):
            nc.tensor.matmul(pAR[:], lhsT=gOH[:, c, :], rhs=Anb[:, c, :],
                             start=(c == 0), stop=(c == 1))
        AnRowsB = cp.tile([S, NODE], BF16)
        nc.vector.tensor_copy(AnRowsB[:], pAR[:])

        if stage == 2:
            nc.sync.dma_start(out_d.rearrange("b 1 -> 1 b"), AnRowsB[0:1, 0:S])
            return

        R = cp.tile([128, 4, S], F32)    # r vectors, feature-major
        if stage:
            nc.vector.memset(R[:], 0.0)

        # ---------------- per-PAIR pipeline ---------------------------------
        for i in range(S // 2):
            b0 = 2 * i
            # one-hots OH[j, m] = (sid[b, j] == m), j on partitions (gpsimd)
            OHp = wp.tile([128, 2, NODE], BF16, tag="OH")
            for s in range(2):
                nc.gpsimd.tensor_scalar(out=OHp[:, s, :], in0=iotaRowF[:],
                                        scalar1=sidF[:, b0 + s:b0 + s + 1],
                                        scalar2=None, op0=OP.is_equal)
            Fbp = wp.tile([128, 2, FEAT], BF16, tag="Fb")
            nc.gpsimd.tensor_copy(Fbp[:], Fall[:, b0:b0 + 2, 0:FEAT])

            # scatter: Xs[m, f] = sum_j OH[j, m] F[j, f]  (node-major)
            Xsbp = wp.tile([128, 2, 2, FEAT], BF16, tag="Xsb")   # [p, s, c, f]
            for s in range(2):
                pXs = ps.tile([128, 2, FEAT], F32, tag="w")
                for c in range(2):
                    nc.tensor.matmul(pXs[:, c, :],
                                     lhsT=OHp[:, s, 128 * c:128 * (c + 1)],
                                     rhs=Fbp[:, s, :], start=True, stop=True)
                if s == 0:
                    nc.vector.tensor_copy(Xsbp[:, s, :, :], pXs[:])
                else:
                    nc.scalar.activation(Xsbp[:, s, :, :], pXs[:], ACTF.Copy)

            # L1 graph-mult: Y1T[f, n] = sum_m Xs[m, f] An[m, n] (feat-major)
            pY1 = ps.tile([96, 2, NODE], F32, tag="w")
            for s in range(2):
                for c in range(2):
                    nc.tensor.matmul(pY1[0:FEAT, s, :], lhsT=Xsbp[:, s, c, :],
                                     rhs=Anb[:, c, :],
                                     start=(c == 0), stop=(c == 1))
            Y1Tp = wp.tile([96, 2, NODE], BF16, tag="Y1T")
            nc.gpsimd.memset(Y1Tp[32:64, :, :], 0.0)
            nc.gpsimd.memset(Y1Tp[64:96, :, :], 0.0)
            nc.scalar.activation(Y1Tp[0:FEAT, :, :], pY1[0:FEAT, :, :], ACTF.Copy)
            nc.gpsimd.memset(Y1Tp[64:65, :, :], 1.0)

            if stage == 31:
                if i == S // 2 - 1:
                    nc.sync.dma_start(out_d.rearrange("b 1 -> 1 b"), Y1Tp[0:1, 0, 0:S])
                continue

            # W1 (+b1 via ones row), relu -> X1 node-major [128, t, s, 512]
            X1bp = wp.tile([128, 2, 2, H1], BF16, tag="X1")
            for s in range(2):
                for t in range(2):
                    pX1 = ps.tile([128, H1], F32, tag="w")
                    nc.tensor.matmul(pX1[:], lhsT=Y1Tp[:, s, 128 * t:128 * (t + 1)],
                                     rhs=W1p[:], start=True, stop=True)
                    if s == 0 and t == 0:
                        nc.vector.tensor_scalar_max(X1bp[:, t, s, :], pX1[:], 0.0)
                    else:
                        nc.scalar.activation(X1bp[:, t, s, :], pX1[:], ACTF.Relu)

            if stage == 32:
                if i == S // 2 - 1:
                    nc.sync.dma_start(out_d.rearrange("b 1 -> 1 b"), X1bp[0:1, 0, 0, 0:S])
                continue

            # L2 graph-mult, feature-major: ZT[h, n] = sum_m X1[m, h] An[m, n]
            ZTbp = wp.tile([128, 4, 2, NODE], BF16, tag="ZT")   # [p, mb, s, n]
            for mb in range(4):
                pZT = ps.tile([128, 2, NODE], F32, tag="w")
                for s in range(2):
                    for k in range(2):
                        nc.tensor.matmul(pZT[:, s, :],
                                         lhsT=X1bp[:, k, s, 128 * mb:128 * (mb + 1)],
                                         rhs=Anb[:, k, :],
                                         start=(k == 0), stop=(k == 1))
                if mb % 2 == 0:
                    nc.scalar.activation(ZTbp[:, mb, :, :], pZT[:], ACTF.Copy)
                else:
                    nc.vector.tensor_copy(ZTbp[:, mb, :, :], pZT[:])

            if stage == 33:
                if i == S // 2 - 1:
                    nc.sync.dma_start(out_d.rearrange("b 1 -> 1 b"), ZTbp[0:1, 0, 0, 0:S])
                continue

            # W2 + b2, relu -> X2T feature-major [p, mb, s, n], N=512 matmuls
            X2bp = wp.tile([128, 4, 2, NODE], BF16, tag="X2")
            for mb in range(4):
                pX2 = ps.tile([128, 2, NODE], F32, tag="w")
                for c in range(4):
                    nc.tensor.matmul(pX2[:],
                                     lhsT=W2sb[:, c, 128 * mb:128 * (mb + 1)],
                                     rhs=ZTbp[:, c, :, :],
                                     start=(c == 0), stop=(c == 3))
                if mb % 2 == 0:
                    nc.scalar.activation(X2bp[:, mb, :, :], pX2[:], ACTF.Relu,
                                         bias=b2c[:, mb:mb + 1])
                else:
                    nc.vector.tensor_scalar(
                        out=X2bp[:, mb, :, :], in0=pX2[:],
                        scalar1=b2c[:, mb:mb + 1], scalar2=0.0,
                        op0=OP.add, op1=OP.max)

            if stage == 34:
                if i == S // 2 - 1:
                    nc.sync.dma_start(out_d.rearrange("b 1 -> 1 b"), X2bp[0:1, 0, 0, 0:S])
                continue

            # r[h] = sum_n X2T[h, n] * An[g_b, n]  (in1 read from PSUM)
            junkV = wp.tile([128, NODE], BF16, tag="junkV")
            for s in range(2):
                b = b0 + s
                pB = ps.tile([128, NODE], F32, tag="w")
                nc.tensor.matmul(pB[:], lhsT=Id32b[:, b:b + 1].to_broadcast([S, 128]),
                                 rhs=AnRowsB[:], start=True, stop=True)
                for m in range(4):
                    nc.vector.scalar_tensor_tensor(
                        out=junkV[:], in0=X2bp[:, m, s, :], scalar=1.0, in1=pB[:],
                        op0=OP.mult, op1=OP.mult, accum_out=R[:, m, b:b + 1])

        if stage == 3 or (30 < stage < 40):
            nc.sync.dma_start(out_d.rearrange("b 1 -> 1 b"), R[0:1, 0, 0:S])
            return

        # ---------------- batched head (feature-major) -----------------------
        Rbb = cp.tile([128, 4, S], BF16)
        nc.vector.tensor_copy(Rbb[:], R[:])
        # G3 = relu(W3^T r + b3), feature-major [128, 8, S]
        G3 = cp.tile([128, 8, S], BF16)
        for mb in range(8):
            pG = ps.tile([128, S], F32, tag="w")
            for c in range(4):
                nc.tensor.matmul(pG[:], lhsT=W3sb[:, c, 128 * mb:128 * (mb + 1)],
                                 rhs=Rbb[:, c, :], start=(c == 0), stop=(c == 3))
            if mb % 2 == 0:
                nc.scalar.activation(G3[:, mb, :], pG[:], ACTF.Relu,
                                     bias=b3c[:, mb:mb + 1])
            else:
                nc.vector.tensor_scalar(
                    out=G3[:, mb, :], in0=pG[:],
                    scalar1=b3c[:, mb:mb + 1], scalar2=0.0,
                    op0=OP.add, op1=OP.max)

        # H = fcW1^T G3, feature-major [128, 4, S] f32 (fcb1 cancelled by BN)
        Hf = cp.tile([128, 4, S], F32)
        for mb in range(4):
            pH = ps.tile([128, S], F32, tag="w")
            for c in range(8):
                nc.tensor.matmul(pH[:], lhsT=fcW1sb[:, c, 128 * mb:128 * (mb + 1)],
                                 rhs=G3[:, c, :], start=(c == 0), stop=(c == 7))
            if mb % 2 == 0:
                nc.scalar.activation(Hf[:, mb, :], pH[:], ACTF.Identity)
            else:
                nc.vector.tensor_copy(Hf[:, mb, :], pH[:])

        # local BN stats: cols 0-3 sums, 4-7 sum-squares
        stats = cp.tile([128, 8], F32)
        sjunk = cp.tile([128, S], F32)
        for m in range(4):
            nc.vector.tensor_reduce(stats[:, m:m + 1], Hf[:, m, :], axis=AX.X,
                                    op=OP.add)
            nc.scalar.activation(sjunk[:], Hf[:, m, :], ACTF.Square,
                                 accum_out=stats[:, 4 + m:5 + m])

        if stage == 4:
            nc.sync.dma_start(out_d.rearrange("b 1 -> 1 b"), stats[0:1, 0:S])
            return

        cc_in = dp.tile([128, 8], F32)
        cc_out = dp.tile([128, 8], F32)
        nc.sync.dma_start(cc_in[:], stats[:])
        nc.gpsimd.collective_compute(
            "AllReduce", OP.add, replica_groups=[list(range(N_CORES))],
            ins=[cc_in.opt()], outs=[cc_out.opt()])
        statsG = cp.tile([128, 8], F32)
        nc.sync.dma_start(statsG[:], cc_out[:])

        if stage == 5:
            nc.sync.dma_start(out_d.rearrange("b 1 -> 1 b"), statsG[0:1, 0:S])
            return

        # BN math, per-partition [128, 4]
        inv_n = 1.0 / (S * N_CORES)
        mean = cp.tile([128, 4], F32)
        nc.vector.tensor_scalar_mul(mean[:], statsG[:, 0:4], inv_n)
        ex2 = cp.tile([128, 4], F32)
        nc.vector.tensor_scalar_mul(ex2[:], statsG[:, 4:8], inv_n)
        var = cp.tile([128, 4], F32)
        nc.vector.tensor_tensor(var[:], mean[:], mean[:], op=OP.mult)
        nc.vector.tensor_tensor(var[:], ex2[:], var[:], op=OP.subtract)
        sd = cp.tile([128, 4], F32)
        nc.scalar.activation(sd[:], var[:], ACTF.Sqrt, bias=epsc[:, 0:1])
        rstd = cp.tile([128, 4], F32)
        nc.vector.reciprocal(rstd[:], sd[:])
        scl = cp.tile([128, 4], F32)
        nc.vector.tensor_tensor(scl[:], gamc[:], rstd[:], op=OP.mult)
        sft = cp.tile([128, 4], F32)
        nc.vector.tensor_tensor(sft[:], mean[:], scl[:], op=OP.mult)
        nc.vector.tensor_tensor(sft[:], betc[:], sft[:], op=OP.subtract)

        # Hl = leaky(H*scl + sft); out = sigmoid(fcW2^T Hl + fcb2)
        Hn = cp.tile([128, 4, S], F32)
        for m in range(4):
            nc.scalar.activation(Hn[:, m, :], Hf[:, m, :], ACTF.Identity,
                                 scale=scl[:, m:m + 1], bias=sft[:, m:m + 1])
        Hl = cp.tile([128, 4, S], BF16)
        nc.vector.scalar_tensor_tensor(
            out=Hl[:], in0=Hn[:], scalar=LEAKY, in1=Hn[:],
            op0=OP.mult, op1=OP.max)
        pO = ps.tile([1, S], F32, tag="w")
        for c in range(4):
            nc.tensor.matmul(pO[:], lhsT=fcW2c[:, c:c + 1], rhs=Hl[:, c, :],
                             start=(c == 0), stop=(c == 3))
        osig = cp.tile([1, S], F32)
        nc.scalar.activation(osig[:], pO[:], ACTF.Sigmoid, bias=fcb2r[:, 0:1])
        nc.sync.dma_start(out_d.rearrange("b 1 -> 1 b"), osig[:])


_NC_CACHE = {}
_LAST_RESULT = None


def _get_nc(S: int):
    if S not in _NC_CACHE:
        _NC_CACHE[S] = build_nc(S)
    return _NC_CACHE[S]


def kernel(**inputs) -> np.ndarray:
    S = BATCH // N_CORES
    nc = _get_nc(S)
    full_x = np.ascontiguousarray(inputs["x"], dtype=np.float32)
    shared = {}
    for k in ("adj_mat", "W1", "b1", "W2", "b2", "W3", "b3", "fcW1", "fcb1",
              "gamma", "beta", "fcW2", "fcb2"):
        shared[k] = np.ascontiguousarray(inputs[k], dtype=np.float32)
    in_maps = []
    for c in range(N_CORES):
        m = dict(shared)
        m["x"] = np.ascontiguousarray(full_x[c * S:(c + 1) * S])
        in_maps.append(m)
    res = bass_utils.run_bass_kernel_spmd(
        nc, in_maps, core_ids=list(range(N_CORES)))
    global _LAST_RESULT
    _LAST_RESULT = res
    out = np.concatenate([res.results[c]["out"] for c in range(N_CORES)], axis=0)
    return out.astype(np.float32)


if __name__ == "__main__":
    print("building...")
    nc = _get_nc(BATCH // N_CORES)
    print("built ok")


# revision 10
# speedup vs baseline: 1.3694x; 1.2920x over previous
"""GCN2 Trainium2 kernel: 3-layer GCN + FC head with BatchNorm, 8-core data-parallel.

Self-contained: hardcodes shapes from the problem spec.
  x [256, 128, 65] f32, adj_mat [256, 256] f32, W1 [63, 512], b1 [512],
  W2 [512, 512], b2 [512], W3 [512, 1024], b3 [1024], fcW1 [1024, 512],
  fcb1 [512], gamma [512], beta [512], fcW2 [512, 1], fcb2 [1] -> out [256, 1]

Sharding: batch 256 -> 32 samples per core on 8 cores; weights/adj replicated.
BatchNorm batch stats all-reduced across cores (one tiny [128,8] AllReduce).

v3 structure (DMA-lean + PE-dense):
  - No indirect DMA / DMA transposes / DRAM scratch: scatter via on-chip
    one-hot matmuls (iota + is_equal; ids unique per sample).
  - Zero-transpose layer chain:
      Xs (node-major) = OH^T F ; Y1T (feat-major) = Xs^T An ;
      X1 (node-major) = relu(Y1T^T W1p) ; ZT (feat-major) = X1^T An ;
      X2T (feat-major) = relu(W2^T ZT + b2) ; r = X2T . An[g,:] (DVE).
  - Samples processed in PAIRS so W2/ZT-consuming matmuls stream N=512
    (weight loads fully hidden; keeps the PE HAM-warm).
  - All PSUM tiles <= 1 bank, single deep pool for lookahead.
  - Evictions spread over Scalar/Vector/GpSimd; fcb1 dropped (BN cancels).
  - Feature-major head + per-partition BN math (no broadcasts, fast
    reciprocal on [128,4]); Lrelu fused via activation alpha.
"""
import os
import sys

if "/opt/trn_rl_repo" not in sys.path:
    sys.path.insert(0, "/opt/trn_rl_repo")

import numpy as np

import concourse.bass as bass
import concourse.mybir as mybir
import concourse.tile as tile
from concourse import bacc, bass_utils
from concourse.masks import make_identity

N_CORES = 8
BATCH, NODE, SEQ, FEAT = 256, 256, 128, 63   # FEAT = feature_num - 1
H1, H2, H3, FC = 512, 512, 1024, 512
BN_EPS = 1e-5
LEAKY = 0.01

F32 = mybir.dt.float32
BF16 = mybir.dt.bfloat16
I32 = mybir.dt.int32
AX = mybir.AxisListType
OP = mybir.AluOpType
ACTF = mybir.ActivationFunctionType


def build_nc(S: int):
    """Build the SPMD kernel for S samples per core."""
    nc = bacc.Bacc("TRN2", target_bir_lowering=False, debug=False,
                   num_devices=N_CORES)

    x_d = nc.dram_tensor("x", [S, SEQ, FEAT + 2], F32, kind="ExternalInput").ap()
    adj_d = nc.dram_tensor("adj_mat", [NODE, NODE], F32, kind="ExternalInput").ap()
    W1_d = nc.dram_tensor("W1", [FEAT, H1], F32, kind="ExternalInput").ap()
    b1_d = nc.dram_tensor("b1", [H1], F32, kind="ExternalInput").ap()
    W2_d = nc.dram_tensor("W2", [H1, H2], F32, kind="ExternalInput").ap()
    b2_d = nc.dram_tensor("b2", [H2], F32, kind="ExternalInput").ap()
    W3_d = nc.dram_tensor("W3", [H2, H3], F32, kind="ExternalInput").ap()
    b3_d = nc.dram_tensor("b3", [H3], F32, kind="ExternalInput").ap()
    fcW1_d = nc.dram_tensor("fcW1", [H3, FC], F32, kind="ExternalInput").ap()
    fcb1_d = nc.dram_tensor("fcb1", [FC], F32, kind="ExternalInput").ap()
    gamma_d = nc.dram_tensor("gamma", [FC], F32, kind="ExternalInput").ap()
    beta_d = nc.dram_tensor("beta", [FC], F32, kind="ExternalInput").ap()
    fcW2_d = nc.dram_tensor("fcW2", [FC, 1], F32, kind="ExternalInput").ap()
    fcb2_d = nc.dram_tensor("fcb2", [1], F32, kind="ExternalInput").ap()
    out_d = nc.dram_tensor("out", [S, 1], F32, kind="ExternalOutput").ap()

    with tile.TileContext(nc) as tc:
        _body(nc, tc, S, x_d, adj_d, W1_d, b1_d, W2_d, b2_d, W3_d, b3_d,
              fcW1_d, gamma_d, beta_d, fcW2_d, fcb2_d, out_d)
    nc.compile()
    return nc


def _body(nc, tc, S, x_d, adj_d, W1_d, b1_d, W2_d, b2_d, W3_d, b3_d,
          fcW1_d, gamma_d, beta_d, fcW2_d, fcb2_d, out_d):
    stage = int(os.environ.get("BISECT_STAGE", "0"))
    with tc.tile_pool(name="const", bufs=1) as cp, \
         tc.tile_pool(name="work", bufs=3) as wp, \
         tc.tile_pool(name="ps", bufs=7, space="PSUM") as ps, \
         tc.tile_pool(name="dram", bufs=1, space="DRAM") as dp:

        # ---------------- input DMAs (plain f32, contiguous runs) -----------
        A0 = cp.tile([128, 2, NODE], F32)        # chunk c = rows 128c..128c+127
        nc.sync.dma_start(A0[:], adj_d.rearrange("(c p) n -> p c n", p=128))
        Fall = cp.tile([128, S, FEAT + 2], F32)  # [seq, sample, feat]
        nc.sync.dma_start(Fall[:], x_d.rearrange("b j f -> j b f"))
        W1f = cp.tile([FEAT, H1], F32)
        nc.sync.dma_start(W1f[:], W1_d[:])
        b2raw = cp.tile([4, 128], F32)
        nc.sync.dma_start(b2raw[:], b2_d.rearrange("(c p) -> c p", p=128))
        b3raw = cp.tile([8, 128], F32)
        nc.sync.dma_start(b3raw[:], b3_d.rearrange("(c p) -> c p", p=128))
        gbraw = cp.tile([4, 3, 128], F32)        # gamma | beta | fcW2
        nc.sync.dma_start(gbraw[:, 0, :], gamma_d.rearrange("(c p) -> c p", p=128))
        nc.sync.dma_start(gbraw[:, 1, :], beta_d.rearrange("(c p) -> c p", p=128))
        nc.sync.dma_start(gbraw[:, 2, :], fcW2_d.rearrange("(c p) 1 -> c p", p=128))
        fcb2r = cp.tile([1, 1], F32)
        nc.sync.dma_start(fcb2r[:], fcb2_d[None, :])
        # bulk weights on the scalar HWDGE ring (overlap with loop)
        W2f = cp.tile([128, 4, H2], F32)
        nc.scalar.dma_start(W2f[:], W2_d.rearrange("(c p) h -> p c h", p=128))
        W3f = cp.tile([128, 4, H3], F32)
        nc.scalar.dma_start(W3f[:], W3_d.rearrange("(c p) h -> p c h", p=128))
        fcW1f = cp.tile([128, 8, FC], F32)
        nc.scalar.dma_start(fcW1f[:], fcW1_d.rearrange("(c p) h -> p c h", p=128))

        # ---------------- on-chip weight casts f32 -> bf16 ------------------
        W1p = cp.tile([96, H1], BF16)
        nc.vector.memset(W1p[:], 0.0)
        nc.vector.tensor_copy(W1p[0:FEAT, :], W1f[:])
        nc.gpsimd.dma_start(W1p[64:65, :], b1_d[None, :])   # tiny cast-DMA
        W2sb = cp.tile([128, 4, H2], BF16)
        nc.vector.tensor_copy(W2sb[:], W2f[:])
        W3sb = cp.tile([128, 4, H3], BF16)
        nc.scalar.activation(W3sb[:], W3f[:], ACTF.Copy)
        fcW1sb = cp.tile([128, 8, FC], BF16)
        nc.scalar.activation(fcW1sb[:], fcW1f[:], ACTF.Copy)

        ident = cp.tile([128, 128], F32)
        make_identity(nc, ident[:])

        # feature-on-partition constants via PE transpose
        def _tr(raw, n, dtype):
            p = ps.tile([128, n], F32, tag="w")
            nc.tensor.transpose(p[:], raw, ident[0:n, 0:n])
            t = cp.tile([128, n], dtype)
            nc.vector.tensor_copy(t[:], p[:])
            return t

        b2c = _tr(b2raw[:], 4, F32)
        b3c = _tr(b3raw[:], 8, F32)
        gamc = _tr(gbraw[:, 0, :], 4, F32)
        betc = _tr(gbraw[:, 1, :], 4, F32)
        fcW2c = _tr(gbraw[:, 2, :], 4, BF16)

        # ---------------- iotas / one-hot helpers ---------------------------
        iotaRowI = cp.tile([128, NODE], I32)
        nc.gpsimd.iota(iotaRowI[:], pattern=[[1, NODE]], base=0,
                       channel_multiplier=0)
        iotaRowF = cp.tile([128, NODE], F32)
        nc.vector.tensor_copy(iotaRowF[:], iotaRowI[:])
        iotaCNI = cp.tile([128, 2], I32)
        nc.gpsimd.iota(iotaCNI[:], pattern=[[128, 2]], base=0,
                       channel_multiplier=1)
        iotaCNF = cp.tile([128, 2], F32)
        nc.vector.tensor_copy(iotaCNF[:], iotaCNI[:])
        e127 = cp.tile([128, 1], F32)   # one-hot of partition 127
        nc.vector.tensor_scalar(out=e127[:], in0=iotaCNF[:, 0:1],
                                scalar1=float(SEQ - 1), scalar2=None,
                                op0=OP.is_equal)
        Id32b = cp.tile([S, S], BF16)
        make_identity(nc, Id32b[:])
        epsc = cp.tile([128, 1], F32)
        nc.gpsimd.memset(epsc[:], BN_EPS)

        # ---------------- adjacency normalization ---------------------------
        # An = diag(dis) (A + I) diag(dis),  dis = 1/sqrt(rowsum(A) + 1)
        dis = cp.tile([128, 2], F32)
        rs = cp.tile([128, 2], F32)
        for c in range(2):
            nc.vector.tensor_reduce(rs[:, c:c + 1], A0[:, c, :], axis=AX.X, op=OP.add)
        sq = cp.tile([128, 2], F32)
        nc.scalar.activation(sq[:], rs[:], ACTF.Sqrt, bias=1.0)
        nc.vector.reciprocal(dis[:], sq[:])
        dis2 = cp.tile([128, 2], F32)
        nc.vector.tensor_tensor(dis2[:], dis[:], dis[:], op=OP.mult)
        Csc = cp.tile([128, 2, NODE], F32)
        for c in range(2):
            nc.vector.tensor_scalar_mul(Csc[:, c, :], A0[:, c, :],
                                        dis[:, c:c + 1])
        Anb = cp.tile([128, 2, NODE], BF16)      # normalized adjacency, bf16
        for cd in range(2):      # destination row chunk
            for cs in range(2):  # source row chunk
                pT = ps.tile([128, 128], F32, tag="w")
                nc.tensor.transpose(pT[:], Csc[:, cs, 128 * cd:128 * (cd + 1)],
                                    ident[:])
                nc.scalar.activation(Anb[:, cd, 128 * cs:128 * (cs + 1)], pT[:],
                                     ACTF.Copy, scale=dis[:, cd:cd + 1])
        diagb = cp.tile([128, 2, NODE], BF16)
        for c in range(2):
            nc.gpsimd.affine_select(
                out=diagb[:, c, :], in_=dis2[:, c:c + 1].to_broadcast([128, NODE]),
                pattern=[[-1, NODE]], compare_op=OP.is_equal, fill=0.0,
                base=128 * c, channel_multiplier=1)
            nc.vector.tensor_tensor(Anb[:, c, :], Anb[:, c, :], diagb[:, c, :],
                                    op=OP.add)

        if stage == 1:
            nc.sync.dma_start(out_d.rearrange("b 1 -> 1 b"), Anb[0:1, 0, 0:S])
            return

        # ---------------- station ids, g-row one-hots ------------------------
        sidF = cp.tile([128, S], F32)
        nc.vector.tensor_copy(sidF[:], Fall[:, :, FEAT:FEAT + 1].rearrange("p b 1 -> p b"))
        pgs = ps.tile([128, S], F32, tag="w")
        nc.tensor.matmul(pgs[:], lhsT=e127[:, 0:1].to_broadcast([128, 128]),
                         rhs=sidF[:], start=True, stop=True)
        gsidB = cp.tile([128, S], F32)
        nc.vector.tensor_copy(gsidB[:], pgs[:])
        gOH = cp.tile([128, 2, S], BF16)         # node-major one-hot of g_sid
        for c in range(2):
            nc.vector.tensor_scalar(out=gOH[:, c, :], in0=gsidB[:],
                                    scalar1=iotaCNF[:, c:c + 1], scalar2=None,
                                    op0=OP.is_equal)
        pAR = ps.tile([S, NODE], F32, tag="w")
        for c in range(2):
            nc.tensor.matmul(pAR[:], lhsT=gOH[:, c, :], rhs=Anb[:, c, :],
                             start=(c == 0), stop=(c == 1))
        AnRowsB = cp.tile([S, NODE], BF16)
        nc.vector.tensor_copy(AnRowsB[:], pAR[:])

        if stage == 2:
            nc.sync.dma_start(out_d.rearrange("b 1 -> 1 b"), AnRowsB[0:1, 0:S])
            return

        R = cp.tile([128, 4, S], F32)    # r vectors, feature-major
        if stage:
            nc.vector.memset(R[:], 0.0)

        # ---------------- per-PAIR pipeline ---------------------------------
        for i in range(S // 2):
            b0 = 2 * i
            # one-hots OH[j, m] = (sid[b, j] == m), j on partitions (gpsimd)
            OHp = wp.tile([128, 2, NODE], BF16, tag="OH")
            for s in range(2):
                nc.vector.tensor_scalar(out=OHp[:, s, :], in0=iotaRowF[:],
                                        scalar1=sidF[:, b0 + s:b0 + s + 1],
                                        scalar2=None, op0=OP.is_equal)
            Fbp = wp.tile([128, 2, FEAT], BF16, tag="Fb")
            nc.gpsimd.tensor_copy(Fbp[:], Fall[:, b0:b0 + 2, 0:FEAT])

            # scatter: Xs[m, f] = sum_j OH[j, m] F[j, f]  (node-major)
            Xsbp = wp.tile([128, 2, 2, FEAT], BF16, tag="Xsb")   # [p, s, c, f]
            for s in range(2):
                pXs = ps.tile([128, 2, FEAT], F32, tag="w")
                for c in range(2):
                    nc.tensor.matmul(pXs[:, c, :],
                                     lhsT=OHp[:, s, 128 * c:128 * (c + 1)],
                                     rhs=Fbp[:, s, :], start=True, stop=True)
                if s == 0:
                    nc.vector.tensor_copy(Xsbp[:, s, :, :], pXs[:])
                else:
                    nc.scalar.activation(Xsbp[:, s, :, :], pXs[:], ACTF.Copy)

            # L1 graph-mult: Y1T[f, n] = sum_m Xs[m, f] An[m, n] (feat-major)
            pY1 = ps.tile([96, 2, NODE], F32, tag="w")
            for s in range(2):
                for c in range(2):
                    nc.tensor.matmul(pY1[0:FEAT, s, :], lhsT=Xsbp[:, s, c, :],
                                     rhs=Anb[:, c, :],
                                     start=(c == 0), stop=(c == 1))
            Y1Tp = wp.tile([96, 2, NODE], BF16, tag="Y1T")
            nc.gpsimd.memset(Y1Tp[32:64, :, :], 0.0)
            nc.gpsimd.memset(Y1Tp[64:96, :, :], 0.0)
            nc.scalar.activation(Y1Tp[0:FEAT, :, :], pY1[0:FEAT, :, :], ACTF.Copy)
            nc.gpsimd.memset(Y1Tp[64:65, :, :], 1.0)

            if stage == 31:
                if i == S // 2 - 1:
                    nc.sync.dma_start(out_d.rearrange("b 1 -> 1 b"), Y1Tp[0:1, 0, 0:S])
                continue

            # W1 (+b1 via ones row), relu -> X1 node-major [128, t, s, 512]
            X1bp = wp.tile([128, 2, 2, H1], BF16, tag="X1")
            for s in range(2):
                for t in range(2):
                    pX1 = ps.tile([128, H1], F32, tag="w")
                    nc.tensor.matmul(pX1[:], lhsT=Y1Tp[:, s, 128 * t:128 * (t + 1)],
                                     rhs=W1p[:], start=True, stop=True)
                    if s == 0 and t == 0:
                        nc.vector.tensor_scalar_max(X1bp[:, t, s, :], pX1[:], 0.0)
                    else:
                        nc.scalar.activation(X1bp[:, t, s, :], pX1[:], ACTF.Relu)

            if stage == 32:
                if i == S // 2 - 1:
                    nc.sync.dma_start(out_d.rearrange("b 1 -> 1 b"), X1bp[0:1, 0, 0, 0:S])
                continue

            # L2 graph-mult, feature-major: ZT[h, n] = sum_m X1[m, h] An[m, n]
            ZTbp = wp.tile([128, 4, 2, NODE], BF16, tag="ZT")   # [p, mb, s, n]
            for mb in range(4):
                pZT = ps.tile([128, 2, NODE], F32, tag="w")
                for s in range(2):
                    for k in range(2):
                        nc.tensor.matmul(pZT[:, s, :],
                                         lhsT=X1bp[:, k, s, 128 * mb:128 * (mb + 1)],
                                         rhs=Anb[:, k, :],
                                         start=(k == 0), stop=(k == 1))
                if mb % 2 == 0:
                    nc.scalar.activation(ZTbp[:, mb, :, :], pZT[:], ACTF.Copy)
                else:
                    nc.vector.tensor_copy(ZTbp[:, mb, :, :], pZT[:])

            if stage == 33:
                if i == S // 2 - 1:
                    nc.sync.dma_start(out_d.rearrange("b 1 -> 1 b"), ZTbp[0:1, 0, 0, 0:S])
                continue

            # W2 + b2, relu -> X2T feature-major [p, mb, s, n], N=512 matmuls
            X2bp = wp.tile([128, 4, 2, NODE], BF16, tag="X2")
            for mb in range(4):
                pX2 = ps.tile([128, 2, NODE], F32, tag="w")
                for c in range(4):
                    nc.tensor.matmul(pX2[:],
                                     lhsT=W2sb[:, c, 128 * mb:128 * (mb + 1)],
                                     rhs=ZTbp[:, c, :, :],
                                     start=(c == 0), stop=(c == 3))
                if mb % 2 == 0:
                    nc.scalar.activation(X2bp[:, mb, :, :], pX2[:], ACTF.Relu,
                                         bias=b2c[:, mb:mb + 1])
                else:
                    nc.vector.tensor_scalar(
                        out=X2bp[:, mb, :, :], in0=pX2[:],
                        scalar1=b2c[:, mb:mb + 1], scalar2=0.0,
                        op0=OP.add, op1=OP.max)

            if stage == 34:
                if i == S // 2 - 1:
                    nc.sync.dma_start(out_d.rearrange("b 1 -> 1 b"), X2bp[0:1, 0, 0, 0:S])
                continue

            # r[h] = sum_n X2T[h, n] * An[g_b, n]  (in1 read from PSUM)
            junkV = wp.tile([128, NODE], BF16, tag="junkV")
            for s in range(2):
                b = b0 + s
                pB = ps.tile([128, NODE], F32, tag="w")
                nc.tensor.matmul(pB[:], lhsT=Id32b[:, b:b + 1].to_broadcast([S, 128]),
                                 rhs=AnRowsB[:], start=True, stop=True)
                for m in range(4):
                    nc.vector.scalar_tensor_tensor(
                        out=junkV[:], in0=X2bp[:, m, s, :], scalar=1.0, in1=pB[:],
                        op0=OP.mult, op1=OP.mult, accum_out=R[:, m, b:b + 1])

        if stage == 3 or (30 < stage < 40):
            nc.sync.dma_start(out_d.rearrange("b 1 -> 1 b"), R[0:1, 0, 0:S])
            return

        # ---------------- batched head (feature-major) -----------------------
        Rbb = cp.tile([128, 4, S], BF16)
        nc.vector.tensor_copy(Rbb[:], R[:])
        # G3 = relu(W3^T r + b3), feature-major [128, 8, S]
        G3 = cp.tile([128, 8, S], BF16)
        for mb in range(8):
            pG = ps.tile([128, S], F32, tag="w")
            for c in range(4):
                nc.tensor.matmul(pG[:], lhsT=W3sb[:, c, 128 * mb:128 * (mb + 1)],
                                 rhs=Rbb[:, c, :], start=(c == 0), stop=(c == 3))
            if mb % 2 == 0:
                nc.scalar.activation(G3[:, mb, :], pG[:], ACTF.Relu,
                                     bias=b3c[:, mb:mb + 1])
            else:
                nc.vector.tensor_scalar(
                    out=G3[:, mb, :], in0=pG[:],
                    scalar1=b3c[:, mb:mb + 1], scalar2=0.0,
                    op0=OP.add, op1=OP.max)

        # H = fcW1^T G3, feature-major [128, 4, S] f32 (fcb1 cancelled by BN)
        Hf = cp.tile([128, 4, S], F32)
        for mb in range(4):
            pH = ps.tile([128, S], F32, tag="w")
            for c in range(8):
                nc.tensor.matmul(pH[:], lhsT=fcW1sb[:, c, 128 * mb:128 * (mb + 1)],
                                 rhs=G3[:, c, :], start=(c == 0), stop=(c == 7))
            if mb % 2 == 0:
                nc.scalar.activation(Hf[:, mb, :], pH[:], ACTF.Identity)
            else:
                nc.vector.tensor_copy(Hf[:, mb, :], pH[:])

        # local BN stats: cols 0-3 sums, 4-7 sum-squares
        stats = cp.tile([128, 8], F32)
        sjunk = cp.tile([128, S], F32)
        for m in range(4):
            nc.vector.tensor_reduce(stats[:, m:m + 1], Hf[:, m, :], axis=AX.X,
                                    op=OP.add)
            nc.scalar.activation(sjunk[:], Hf[:, m, :], ACTF.Square,
                                 accum_out=stats[:, 4 + m:5 + m])

        if stage == 4:
            nc.sync.dma_start(out_d.rearrange("b 1 -> 1 b"), stats[0:1, 0:S])
            return

        cc_in = dp.tile([128, 8], F32)
        cc_out = dp.tile([128, 8], F32)
        nc.sync.dma_start(cc_in[:], stats[:])
        nc.gpsimd.collective_compute(
            "AllReduce", OP.add, replica_groups=[list(range(N_CORES))],
            ins=[cc_in.opt()], outs=[cc_out.opt()])
        statsG = cp.tile([128, 8], F32)
        nc.sync.dma_start(statsG[:], cc_out[:])

        if stage == 5:
            nc.sync.dma_start(out_d.rearrange("b 1 -> 1 b"), statsG[0:1, 0:S])
            return

        # BN math, per-partition [128, 4]
        inv_n = 1.0 / (S * N_CORES)
        mean = cp.tile([128, 4], F32)
        nc.vector.tensor_scalar_mul(mean[:], statsG[:, 0:4], inv_n)
        ex2 = cp.tile([128, 4], F32)
        nc.vector.tensor_scalar_mul(ex2[:], statsG[:, 4:8], inv_n)
        var = cp.tile([128, 4], F32)
        nc.vector.tensor_tensor(var[:], mean[:], mean[:], op=OP.mult)
        nc.vector.tensor_tensor(var[:], ex2[:], var[:], op=OP.subtract)
        sd = cp.tile([128, 4], F32)
        nc.scalar.activation(sd[:], var[:], ACTF.Sqrt, bias=epsc[:, 0:1])
        rstd = cp.tile([128, 4], F32)
        nc.vector.reciprocal(rstd[:], sd[:])
        scl = cp.tile([128, 4], F32)
        nc.vector.tensor_tensor(scl[:], gamc[:], rstd[:], op=OP.mult)
        sft = cp.tile([128, 4], F32)
        nc.vector.tensor_tensor(sft[:], mean[:], scl[:], op=OP.mult)
        nc.vector.tensor_tensor(sft[:], betc[:], sft[:], op=OP.subtract)

        # Hl = leaky(H*scl + sft); out = sigmoid(fcW2^T Hl + fcb2)
        Hn = cp.tile([128, 4, S], F32)
        for m in range(4):
            nc.scalar.activation(Hn[:, m, :], Hf[:, m, :], ACTF.Identity,
                                 scale=scl[:, m:m + 1], bias=sft[:, m:m + 1])
        Hl = cp.tile([128, 4, S], BF16)
        nc.vector.scalar_tensor_tensor(
            out=Hl[:], in0=Hn[:], scalar=LEAKY, in1=Hn[:],
            op0=OP.mult, op1=OP.max)
        pO = ps.tile([1, S], F32, tag="w")
        for c in range(4):
            nc.tensor.matmul(pO[:], lhsT=fcW2c[:, c:c + 1], rhs=Hl[:, c, :],
                             start=(c == 0), stop=(c == 3))
        osig = cp.tile([1, S], F32)
        nc.scalar.activation(osig[:], pO[:], ACTF.Sigmoid, bias=fcb2r[:, 0:1])
        nc.sync.dma_start(out_d.rearrange("b 1 -> 1 b"), osig[:])


_NC_CACHE = {}
_LAST_RESULT = None


def _get_nc(S: int):
    if S not in _NC_CACHE:
        _NC_CACHE[S] = build_nc(S)
    return _NC_CACHE[S]


def kernel(**inputs) -> np.ndarray:
    S = BATCH // N_CORES
    nc = _get_nc(S)
    full_x = np.ascontiguousarray(inputs["x"], dtype=np.float32)
    shared = {}
    for k in ("adj_mat", "W1", "b1", "W2", "b2", "W3", "b3", "fcW1", "fcb1",
              "gamma", "beta", "fcW2", "fcb2"):
        shared[k] = np.ascontiguousarray(inputs[k], dtype=np.float32)
    in_maps = []
    for c in range(N_CORES):
        m = dict(shared)
        m["x"] = np.ascontiguousarray(full_x[c * S:(c + 1) * S])
        in_maps.append(m)
    res = bass_utils.run_bass_kernel_spmd(
        nc, in_maps, core_ids=list(range(N_CORES)))
    global _LAST_RESULT
    _LAST_RESULT = res
    out = np.concatenate([res.results[c]["out"] for c in range(N_CORES)], axis=0)
    return out.astype(np.float32)


if __name__ == "__main__":
    print("building...")
    nc = _get_nc(BATCH // N_CORES)
    print("built ok")
